# revision 6
# baseline (speedup 1.0000x reference)
"""Trainium2 Bass kernel for BeatDetectionRSNN2 (2-layer integrate-and-fire RSNN).

Reference semantics (per time step t):
    v1 += x_t @ W1.T ; s1 = (v1 >= 1); v1 *= (1 - s1)
    v2 += s1 @ W2.T  ; s2 = (v2 >= 1); v2 *= (1 - s2)
    out[:, t, :] = s2

Sharding: data-parallel over batch across 8 cores (16 batch rows each),
weights replicated, time recurrence local per core.

Per-core design (B_c=16, T=t_steps, D=128, O=2):
  - The serial recurrence is the critical path: 2 dependent DVE ops per
    step on a [128, 17] tile (cols 0..15 = v1 as [d, b], col 16 = v2 for
    the 32 (o,b) pairs on partitions 0..31, fused into the same ops with
    a LAG=64-step delay).  Dependent-op latency ~173ns -> ~347ns/step.
  - Everything else leads/trails the chain in 32-step groups on other
    engines: PE (x transposes, u1 = x@W1.T, u2 = s1@W2.T), ACT (psum->
    sbuf copies), Pool (spike extraction + small DMAs), SP (x DMAs).
  - x arrives as ONE DMA per 128-step chunk into a [t, (b d)] staging
    tile (16 separate DMAs would serialize ~10us on the HWDGE).
  - u2 for times [g*32, g*32+32) is produced right after the chain
    passes group g (extract -> matmul -> copy -> remap-DMA -> copy,
    ~6us) and consumed at chain steps +64 (one full group of slack, so
    the chain never stalls).  The tail re-runs only 64 layer-2-only
    steps ([32,1] op pairs) instead of 256.
"""
import sys
import numpy as np

if '/opt/trn_rl_repo' not in sys.path:
    sys.path.insert(0, '/opt/trn_rl_repo')

import concourse.bacc as bacc
import concourse.tile as tile
import concourse.mybir as mybir
from concourse.masks import make_identity
from concourse.bass_utils import run_bass_kernel_spmd

f32 = mybir.dt.float32
Alu = mybir.AluOpType

B, T, D, O = 128, 4096, 128, 2
NCORES = 8
BC = B // NCORES          # 16 batch rows per core
K = 128                   # chunk (time steps): x DMA / transpose granularity
G = 32                    # group (time steps): chain <-> layer-2 pipelining
GPC = K // G              # groups per chunk
FD = BC + 1               # 17 chain columns
LAG = G + G // 2          # layer-2 trails layer-1 by 48 steps


def build_program(t_steps=T):
    assert t_steps % K == 0 and t_steps >= 2 * K
    nch = t_steps // K
    ngrp = t_steps // G
    nc = bacc.Bacc("TRN2", target_bir_lowering=False)
    x_ext = nc.declare_dram_parameter("x", [BC, t_steps, D], f32, isOutput=False)
    w1t_ext = nc.declare_dram_parameter("w1t", [D, D], f32, isOutput=False)
    w2t_ext = nc.declare_dram_parameter("w2t", [D, O], f32, isOutput=False)
    # output stored (o, b, t): per-group DMA from [32, G] staging (partition
    # p = 16*o + b) is contiguous; host transposes to [b, t, o].
    out_ext = nc.declare_dram_parameter("out", [O, BC, t_steps], f32, isOutput=True)

    with tile.TileContext(nc) as tc:
        with (
            tc.tile_pool(name="consts", bufs=1) as consts,
            tc.tile_pool(name="xst", bufs=3) as xst_pool,
            tc.tile_pool(name="xT", bufs=2) as xT_pool,
            tc.tile_pool(name="ubuf", bufs=10) as u_pool,
            tc.tile_pool(name="wbuf", bufs=6) as w_pool,
            tc.tile_pool(name="gbuf", bufs=3) as g_pool,
            tc.tile_pool(name="s2st", bufs=3) as s2_pool,
            tc.tile_pool(name="u2sb", bufs=3) as u2sb_pool,
            tc.tile_pool(name="u2c", bufs=3) as u2c_pool,
            tc.tile_pool(name="tailu", bufs=1) as tail_pool,
            tc.tile_pool(name="xpose", bufs=2, space="PSUM") as xpose_pool,
            tc.tile_pool(name="upsum", bufs=2, space="PSUM") as upsum_pool,
            tc.tile_pool(name="u2psum", bufs=2, space="PSUM") as u2psum_pool,
        ):
            ident = consts.tile([128, 128], f32)
            make_identity(nc, ident[:])
            w1t = consts.tile([D, D], f32)
            w2t = consts.tile([D, O], f32)
            v_all = consts.tile([128, FD], f32)
            nc.sync.dma_start(w1t[:], w1t_ext[:])
            nc.sync.dma_start(w2t[:], w2t_ext[:])
            nc.vector.memset(v_all[:], 0.0)

            # u tiles: one per group, [128, FD*G]; col f=16 = layer-2 input
            # (times shifted back by LAG), cols 0..15 = u1 laid out [d, b].
            u_tiles = [u_pool.tile([128, FD * G], f32, tag="ubuf", name=f"u_g{g}")
                       for g in range(ngrp)]
            # layer-2 input for the first LAG chain steps is zero (v2 idle)
            nc.vector.memset(u_tiles[0][0:32, BC::FD], 0.0)
            nc.vector.memset(u_tiles[1][0:32, BC:BC + 16 * FD:FD], 0.0)
            # tail u2: layer-2 inputs for times [T-LAG, T)
            tail_u2 = tail_pool.tile([32, LAG], f32, tag="tailu")

            def dma_x_chunk(c, fine=False):
                """One DMA: x[:, cK:(c+1)K, :] -> staging [t, (b d)]."""
                rows = G if fine else K
                xs = xst_pool.tile([rows, BC * D], f32, tag="xst",
                                   name=f"xs{c}" + ("f" if fine else ""))
                src = x_ext[:, c * K:c * K + rows, :].rearrange("b t d -> t b d")
                nc.sync.dma_start(xs[:].rearrange("t (b d) -> t b d", b=BC), src)
                return xs

            def transpose_chunk(c, xs, xT, fine_skip_tg0=False):
                """PE-transpose staging [t, (b d)] -> xT [d, (tg b tlo)]."""
                tg_lo = 1 if fine_skip_tg0 else 0
                for j in range(4):
                    xp = xpose_pool.tile([128, 4, 128], f32, tag="xpose")
                    for i in range(4):
                        b = 4 * j + i
                        nc.tensor.transpose(xp[:, i, :], xs[:, b * D:(b + 1) * D],
                                            ident[:])
                    src = xp[:].rearrange("p b (tg tlo) -> p tg b tlo", tg=GPC)
                    dst = xT[:].rearrange("p (tg b tlo) -> p tg b tlo",
                                          tg=GPC, b=BC)[:, tg_lo:, 4 * j:4 * j + 4, :]
                    nc.scalar.copy(dst, src[:, tg_lo:, :, :])

            def u1_matmul(g, xT, tg, split=1):
                """u1 for group g from xT cols [tg*512, (tg+1)*512)."""
                TW = G // split
                blk = xT[:, tg * BC * G:(tg + 1) * BC * G].rearrange(
                    "p (b t) -> p b t", b=BC)
                for s in range(split):
                    up = upsum_pool.tile([128, BC * TW], f32, tag="upsum")
                    nc.tensor.matmul(up[:], w1t[:], blk[:, :, s * TW:(s + 1) * TW],
                                     start=True, stop=True)
                    dst = u_tiles[g][:].rearrange("p (t f) -> p f t", f=FD)[
                        :, 0:BC, s * TW:(s + 1) * TW]
                    src = up[:].rearrange("p (b t) -> p b t", b=BC)
                    nc.scalar.copy(dst, src)

            # ---- prologue: chunk 0 (fine-grained first group) ----
            xsf = dma_x_chunk(0, fine=True)
            xs0 = dma_x_chunk(0)
            xT0 = xT_pool.tile([128, BC * K], f32, tag="xT", name="xT0")
            xpf = xpose_pool.tile([128, BC * G], f32, tag="xpose")
            for b in range(BC):
                nc.tensor.transpose(xpf[:, b * G:(b + 1) * G],
                                    xsf[:, b * D:(b + 1) * D], ident[0:G, 0:G])
            nc.scalar.copy(xT0[:, 0:BC * G], xpf[:])
            u1_matmul(0, xT0, 0, split=4)
            transpose_chunk(0, xs0, xT0, fine_skip_tg0=True)
            for tg in range(1, GPC):
                u1_matmul(tg, xT0, tg)

            xT_next = None
            for c in range(nch):
                if c + 1 < nch:
                    xs = dma_x_chunk(c + 1)
                    xT_next = xT_pool.tile([128, BC * K], f32, tag="xT",
                                           name=f"xT{c + 1}")
                    transpose_chunk(c + 1, xs, xT_next)
                for tg in range(GPC):
                    g = c * GPC + tg
                    u_t = u_tiles[g]
                    w_t = w_pool.tile([128, FD * G], f32, tag="wbuf", name=f"w_g{g}")
                    # ---- serial chain: G steps ----
                    for t in range(G):
                        sl = slice(t * FD, t * FD + FD)
                        nc.vector.tensor_tensor(out=w_t[:, sl], in0=v_all[:],
                                                in1=u_t[:, sl], op=Alu.add)
                        nc.vector.scalar_tensor_tensor(out=v_all[:], in0=w_t[:, sl],
                                                       scalar=1.0, in1=w_t[:, sl],
                                                       op0=Alu.is_lt, op1=Alu.mult)

                    # ---- s1 spikes -> u2 for times [g*G, g*G+G) ----
                    g_t = g_pool.tile([128, BC * G], f32, tag="gbuf")
                    g3 = g_t[:].rearrange("p (b t) -> p b t", b=BC)
                    w3 = w_t[:].rearrange("p (t f) -> p f t", f=FD)[:, 0:BC, :]
                    nc.gpsimd.tensor_scalar(g3, w3, 1.0, None, op0=Alu.is_ge)
                    u2p = u2psum_pool.tile([2, BC * G], f32, tag="u2psum")
                    nc.tensor.matmul(u2p[:], w2t[:], g_t[:], start=True, stop=True)
                    # interleave next chunk's u1 matmul behind the u2 matmul
                    # so each u2 matmul sees an idle PE when its inputs land
                    if c + 1 < nch:
                        u1_matmul((c + 1) * GPC + tg, xT_next, tg)
                    u2s = u2sb_pool.tile([2, BC * G], f32, tag="u2sb")
                    nc.scalar.copy(u2s[:], u2p[:])
                    # remap [2, (b t)] -> [32 (p=16o+b), t] (same linear order);
                    # issued from Pool: the SP/HWDGE path would queue behind
                    # the x DMAs
                    u2c = u2c_pool.tile([32, G], f32, tag="u2c")
                    nc.gpsimd.dma_start(u2c[:], u2s[:])
                    # production covers times [32g, 32g+32); u tile h holds
                    # times [32h-LAG, 32h-LAG+G)
                    H = G // 2
                    if g + 1 < ngrp:
                        nc.scalar.copy(
                            u_tiles[g + 1][0:32, BC + H * FD::FD], u2c[:, 0:H])
                    else:
                        lo = g * G - (t_steps - LAG)
                        nc.scalar.copy(tail_u2[:, lo:lo + H], u2c[:, 0:H])
                    if g + 2 < ngrp:
                        nc.scalar.copy(
                            u_tiles[g + 2][0:32, BC:BC + H * FD:FD], u2c[:, H:G])
                    else:
                        lo = g * G + H - (t_steps - LAG)
                        nc.scalar.copy(tail_u2[:, lo:lo + H], u2c[:, H:G])

                    # ---- s2 output for times [g*G - LAG, g*G - LAG + G) ----
                    if g >= 2:
                        s2 = s2_pool.tile([32, G], f32, tag="s2st")
                        nc.gpsimd.tensor_scalar(s2[:], w_t[0:32, BC::FD], 1.0, None,
                                                op0=Alu.is_ge)
                        lo = g * G - LAG
                        nc.gpsimd.dma_start(out_ext[:, :, lo:lo + G], s2[:])
                    elif g == 1:
                        # times [-16, 16): only slots 16..31 are valid
                        H = G // 2
                        s2 = s2_pool.tile([32, H], f32, tag="s2st")
                        nc.gpsimd.tensor_scalar(s2[:], w_t[0:32, BC + H * FD::FD],
                                                1.0, None, op0=Alu.is_ge)
                        nc.gpsimd.dma_start(out_ext[:, :, 0:H], s2[:])

            # ---- tail: layer-2 only, times [T-LAG, T) ----
            wep = tail_pool.tile([32, LAG], f32, tag="tailw")
            for t in range(LAG):
                nc.vector.tensor_tensor(out=wep[:, t:t + 1], in0=v_all[0:32, BC:BC + 1],
                                        in1=tail_u2[:, t:t + 1], op=Alu.add)
                nc.vector.scalar_tensor_tensor(out=v_all[0:32, BC:BC + 1],
                                               in0=wep[:, t:t + 1], scalar=1.0,
                                               in1=wep[:, t:t + 1],
                                               op0=Alu.is_lt, op1=Alu.mult)
            s2 = s2_pool.tile([32, LAG], f32, tag="s2st")
            nc.gpsimd.tensor_scalar(s2[:], wep[:], 1.0, None, op0=Alu.is_ge)
            nc.gpsimd.dma_start(out_ext[:, :, t_steps - LAG:t_steps], s2[:])

    nc.compile()
    return nc


_program_cache = {}


def kernel(x, W1, W2):
    x = np.ascontiguousarray(np.asarray(x, dtype=np.float32))
    W1 = np.asarray(W1, dtype=np.float32)
    W2 = np.asarray(W2, dtype=np.float32)
    t_steps = x.shape[1]
    if t_steps not in _program_cache:
        _program_cache[t_steps] = build_program(t_steps)
    nc = _program_cache[t_steps]

    w1t = np.ascontiguousarray(W1.T)            # [d_in, d_out]
    w2t = np.ascontiguousarray(W2.T)            # [d_in, 2]
    in_maps = [
        {"x": np.ascontiguousarray(x[i * BC:(i + 1) * BC]), "w1t": w1t, "w2t": w2t}
        for i in range(NCORES)
    ]
    res = run_bass_kernel_spmd(nc, in_maps, list(range(NCORES)))
    # device layout is [O, BC, T]; full output is [B, T, O]
    outs = [np.transpose(np.asarray(res.results[i]["out"]), (1, 2, 0))
            for i in range(NCORES)]
    return np.ascontiguousarray(np.concatenate(outs, axis=0)).astype(np.float32)
